# revision 39
# baseline (speedup 1.0000x reference)
"""DogDetector NMS kernel for Trainium2 (Bass/Tile), 8-core data-parallel.

Algorithm (per core, 4 images):
  - gpsimd InstTopk: exact top-256 (values + indices) per 51200-anchor
    half-vocab per image (TOPK ISA n field is u16); the global top-128 of
    an image is contained in the union of its two halves' top-128s
  - exact descending sort of that 256-pool by (conf, -index): pairwise
    compare matrix reduced on DVE for ranks, permutation matmul on PE
    (reproduces jax.lax.top_k tie ordering)
  - per image, gather bbox deltas + anchors of the top-128 with 256B-block
    dma_gathers (bbox_pred is never fully read)
  - decode boxes, IoU matrix [128,128], greedy NMS as a Jacobi fixed
    point iterated on PE, prefix-sum compaction matmul -> [100,5]
"""

import sys

import numpy as np

sys.path.insert(0, "/opt/trn_rl_repo")

import concourse.bacc as bacc
import concourse.bass_isa as bass_isa
import concourse.mybir as mybir
import concourse.tile as tile

B = 32
N = 102400
NCORES = 8
IMGS = B // NCORES          # 4 images per core
P = 128
HV = N // 2                 # half-vocab per topk call
POOL = 256                  # candidate pool (2 x topk half top-128)
NMS_ITERS = 4               # Jacobi iterations (empirical max 3)
MAXDET = 100
CLIP = 4.135

f32 = mybir.dt.float32
i32 = mybir.dt.int32
i16 = mybir.dt.int16
u32 = mybir.dt.uint32
OP = mybir.AluOpType
AF = mybir.ActivationFunctionType
AX = mybir.AxisListType

# host-built constant block [128, CW] f32; column layout:
C_ID = 0        # identity [128,128]
C_UTI = 128     # upper-tri incl diag: [i,j]=1 iff i<=j (row 0 = all ones)
C_UTS = 256     # upper-tri strict:    [i,j]=1 iff i<j
C_I128 = 384    # row 0..127 replicated on all partitions
C_I16 = 512     # row 0..15 replicated
C_M16 = 528     # [r, m] = 1 iff r%16 == m%16   (wrapped-index selection)
C_GRP = 656     # [r, c] = 1 iff r//16 == c     (c = 0..7)
C_LTI = 664     # [k, j] = 1 iff k >= j (lower-tri incl)
C_BID = 792     # 1e30 * identity
CW = 920


def make_consts():
    c = np.zeros((P, CW), np.float32)
    i = np.arange(P)
    c[:, C_ID:C_ID + P] = np.eye(P, dtype=np.float32)
    c[:, C_UTI:C_UTI + P] = (i[:, None] <= i[None, :]).astype(np.float32)
    c[:, C_UTS:C_UTS + P] = (i[:, None] < i[None, :]).astype(np.float32)
    c[:, C_I128:C_I128 + P] = i[None, :].astype(np.float32)
    c[:, C_I16:C_I16 + 16] = np.arange(16, dtype=np.float32)[None, :]
    c[:, C_M16:C_M16 + P] = (i[:, None] % 16 == i[None, :] % 16).astype(np.float32)
    c[:, C_GRP:C_GRP + 8] = (i[:, None] // 16 == np.arange(8)[None, :]).astype(
        np.float32)
    c[:, C_LTI:C_LTI + P] = (i[:, None] >= i[None, :]).astype(np.float32)
    c[:, C_BID:C_BID + P] = 1e30 * np.eye(P, dtype=np.float32)
    return c


def _acopy(nc, out, in_):
    nc.scalar.activation(out, in_, AF.Copy)


def build_kernel(tc, conf, bbox, anch, cst, out):
    nc = tc.nc
    with (
        tc.tile_pool(name="consts", bufs=1) as consts,
        tc.tile_pool(name="confp", bufs=1) as confp,
        tc.tile_pool(name="work", bufs=4) as work,
        tc.tile_pool(name="pp", bufs=2, space="PSUM") as pp,
        tc.tile_pool(name="tailp", bufs=1, space="PSUM") as tailp,
        tc.tile_pool(name="tpsp", bufs=2, space="PSUM") as tpsp,
        tc.tile_pool(name="brpsp", bufs=2, space="PSUM") as brpsp,
        tc.tile_pool(name="bb", bufs=1, space="PSUM") as bb,
    ):
        C = consts.tile([P, CW], f32)
        ident = C[:, C_ID:C_ID + P]
        ut_incl = C[:, C_UTI:C_UTI + P]
        ut_strict = C[:, C_UTS:C_UTS + P]
        iota128c = C[:, C_I128:C_I128 + P]
        iota16c = C[:, C_I16:C_I16 + 16]
        mod16eq = C[:, C_M16:C_M16 + P]
        grp8 = C[:, C_GRP:C_GRP + 8]
        lt_incl = C[:, C_LTI:C_LTI + P]
        bigid = C[:, C_BID:C_BID + P]
        ones1 = C[0:1, C_UTI:C_UTI + P]         # row 0 of ut_incl is all ones

        # ---- stage A: topk per half-vocab, batched row layouts ----
        rowv = []   # per half: [1, 1024] pool values (img-major, ascending)
        rowi = []   # per half: [1, 1024] global anchor indices (f32)
        # one topk call, 8 tokens: tokens 0..3 = half A of each image,
        # tokens 4..7 = half B (engages all 8 Q7 cores at once)
        ctile = confp.tile([P, HV // 16], f32, tag="conf")
        for h in range(2):
            nc.sync.dma_start(
                ctile[64 * h:64 * (h + 1), :],
                conf[:, h * HV:(h + 1) * HV].rearrange("i (p c) -> i p c", p=16))
        nc.sync.dma_start(C[:], cst)
        tk = confp.tile([P, 32], u32, tag="tk")
        nc.gpsimd.add_instruction(bass_isa.InstTopk(
            name=f"I-{nc.next_id()}",
            ins=[nc.gpsimd.lower_ap(ctile[:], for_isa=True)],
            outs=[nc.gpsimd.lower_ap(tk[:], for_isa=True)],
            _tokens=2 * IMGS, _n=HV, _k=POOL))
        ti = confp.tile([P, 16], f32, tag="tkif")
        nc.vector.tensor_copy(ti[:], tk[:, 16:32])
        nc.vector.tensor_scalar(ti[64:P, :], ti[64:P, :], float(HV), None,
                                op0=OP.add)
        for h in range(2):
            rv = confp.tile([1, 1024], f32, tag=f"rowv{h}")
            nc.sync.dma_start(rv[:], tk[64 * h:64 * (h + 1), 0:16].bitcast(f32))
            ri = confp.tile([1, 1024], f32, tag=f"rowi{h}")
            nc.sync.dma_start(ri[:], ti[64 * h:64 * (h + 1), :])
            rowv.append(rv)
            rowi.append(ri)

        outall = work.tile([P, 5 * IMGS], f32, tag="outall")

        # ---- phase-major emission: all images phase 1, then phase 2, ... ----
        def phase1(img):
            """rows bcast + vi columns + exact ranks + permutation sort."""
            sl = slice(POOL * img + P, POOL * img + 2 * P)
            rA, rB = rowv[0][0:1, sl], rowv[1][0:1, sl]
            iA, iB = rowi[0][0:1, sl], rowi[1][0:1, sl]

            vrow_bm = work.tile([P, POOL], f32, tag="vrbm")
            irow_bm = work.tile([P, POOL], f32, tag="irbm")
            for dst, (sa, sb) in ((vrow_bm, (rA, rB)), (irow_bm, (iA, iB))):
                t_ps = bb.tile([P, POOL], f32, tag="bb")
                nc.tensor.matmul(t_ps[:, 0:P], lhsT=ones1, rhs=sa, start=True,
                                 stop=True)
                nc.tensor.matmul(t_ps[:, P:POOL], lhsT=ones1, rhs=sb,
                                 start=True, stop=True)
                _acopy(nc, dst[:], t_ps[:])

            vi = work.tile([P, 4], f32, tag="vi")   # val A, val B, idx A, idx B
            vit_ps = pp.tile([P, 4], f32, tag="pp")
            for col, rsrc in enumerate((rA, rB, iA, iB)):
                nc.tensor.transpose(vit_ps[:, col:col + 1], rsrc,
                                    ident[0:1, 0:1])
            _acopy(nc, vi[:], vit_ps[:])

            rcol = work.tile([P, 2], f32, tag="rcol")
            for blk in range(2):
                rg = work.tile([P, 2], f32, tag="rg")   # gt count, tie count
                a = work.tile([P, POOL], f32, tag="rgt")
                nc.vector.tensor_scalar(a[:], vrow_bm[:], vi[:, blk:blk + 1],
                                        0.0, op0=OP.is_gt, op1=OP.add,
                                        accum_out=rg[:, 0:1])
                b = work.tile([P, POOL], f32, tag="req")
                nc.vector.tensor_scalar(b[:], vrow_bm[:], vi[:, blk:blk + 1],
                                        None, op0=OP.is_equal)
                c_ = work.tile([P, POOL], f32, tag="ril")
                nc.vector.scalar_tensor_tensor(
                    c_[:], irow_bm[:], vi[:, 2 + blk:3 + blk], b[:],
                    op0=OP.is_lt, op1=OP.mult, accum_out=rg[:, 1:2])
                nc.vector.tensor_tensor(rcol[:, blk:blk + 1], rg[:, 0:1],
                                        rg[:, 1:2], op=OP.add)

            srt_ps = pp.tile([P, 2], f32, tag="pp")
            for blk in range(2):
                pt = work.tile([P, P], f32, tag="pt")
                nc.vector.tensor_scalar(pt[:], iota128c, rcol[:, blk:blk + 1],
                                        None, op0=OP.is_equal)
                nc.tensor.matmul(srt_ps[:], lhsT=pt[:], rhs=vi[:, blk:4:2],
                                 start=(blk == 0), stop=(blk == 1))
            srt = work.tile([P, 2], f32, tag="srt")
            nc.vector.tensor_copy(srt[:], srt_ps[:])
            return srt

        def phase2(img, srt):
            """gather indices + the two dma_gathers."""
            ai = work.tile([P, 1], i32, tag="ai")
            nc.vector.tensor_copy(ai[:], srt[:, 1:2])
            blocki = work.tile([P, 1], i32, tag="blocki")
            nc.vector.tensor_scalar(blocki[:], ai[:], 4, None,
                                    op0=OP.logical_shift_right)
            withini = work.tile([P, 1], i32, tag="withini")
            nc.vector.tensor_scalar(withini[:], ai[:], 15, None,
                                    op0=OP.bitwise_and)
            withinf = work.tile([P, 1], f32, tag="withinf")
            nc.vector.tensor_copy(withinf[:], withini[:])
            blockf = work.tile([P, 1], f32, tag="blockf")
            nc.vector.tensor_copy(blockf[:], blocki[:])

            rhs8 = work.tile([P, 8], f32, tag="rhs8")
            nc.vector.tensor_tensor(rhs8[:], blockf[:].broadcast_to([P, 8]),
                                    grp8, op=OP.mult)
            wrap_ps = pp.tile([P, 8], f32, tag="pp")
            nc.tensor.matmul(wrap_ps[:], lhsT=mod16eq, rhs=rhs8[:], start=True,
                             stop=True)
            widx = work.tile([P, 8], i16, tag="widx")
            nc.vector.tensor_copy(widx[:], wrap_ps[:])

            gbox = work.tile([P, 1, 64], f32, tag="gbox")
            nc.gpsimd.dma_gather(
                gbox[:], bbox[img].rearrange("(b k) c -> b (k c)", k=16),
                widx[:], num_idxs=P, num_idxs_reg=P, elem_size=64,
                queue_num=img % 2)
            ganch = work.tile([P, 1, 64], f32, tag="ganch")
            nc.gpsimd.dma_gather(
                ganch[:], anch.rearrange("(b k) c -> b (k c)", k=16),
                widx[:], num_idxs=P, num_idxs_reg=P, elem_size=64,
                queue_num=(img + 1) % 2)
            return srt, withinf, gbox, ganch

        def phase3(img, srt, withinf, gbox, ganch):
            """extract gathered rows + box decode."""
            em = work.tile([P, 16], f32, tag="em")
            nc.vector.tensor_scalar(em[:], iota16c, withinf[:, 0:1], None,
                                    op0=OP.is_equal)
            dd = {}
            for key, gt_ in (("d", gbox), ("a", ganch)):
                pb = work.tile([P, 16, 4], f32, tag="pb")
                nc.vector.tensor_tensor(
                    pb[:], gt_[:, 0, :].rearrange("p (b c) -> p b c", c=4),
                    em[:].rearrange("p (b o) -> p b o", o=1)
                        .broadcast_to([P, 16, 4]),
                    op=OP.mult)
                t4 = work.tile([P, 4], f32, tag=f"ex{key}")
                nc.vector.tensor_reduce(t4[:],
                                        pb[:].rearrange("p b c -> p c b"),
                                        axis=AX.X, op=OP.add)
                dd[key] = t4
            dl, aa = dd["d"], dd["a"]

            wh = work.tile([P, 2], f32, tag="wh")
            nc.vector.tensor_tensor(wh[:], aa[:, 2:4], aa[:, 0:2],
                                    op=OP.subtract)
            ctr = work.tile([P, 2], f32, tag="ctr")
            nc.vector.tensor_scalar(ctr[:], wh[:], 0.5, None, op0=OP.mult)
            nc.vector.tensor_tensor(ctr[:], ctr[:], aa[:, 0:2], op=OP.add)
            dwh = work.tile([P, 2], f32, tag="dwh")
            nc.vector.tensor_scalar(dwh[:], dl[:, 2:4], -CLIP, CLIP,
                                    op0=OP.max, op1=OP.min)
            ewh = work.tile([P, 2], f32, tag="ewh")
            nc.scalar.activation(ewh[:], dwh[:], AF.Exp)
            pwh = work.tile([P, 2], f32, tag="pwh")
            nc.vector.tensor_tensor(pwh[:], ewh[:], wh[:], op=OP.mult)
            pc = work.tile([P, 2], f32, tag="pc")
            nc.vector.tensor_tensor(pc[:], dl[:, 0:2], wh[:], op=OP.mult)
            nc.vector.tensor_tensor(pc[:], pc[:], ctr[:], op=OP.add)
            hwh = work.tile([P, 2], f32, tag="hwh")
            nc.vector.tensor_scalar(hwh[:], pwh[:], 0.5, None, op0=OP.mult)
            bx = work.tile([P, 4], f32, tag="bx")
            nc.vector.tensor_tensor(bx[:, 0:2], pc[:], hwh[:], op=OP.subtract)
            nc.vector.tensor_tensor(bx[:, 2:4], pc[:], hwh[:], op=OP.add)
            wd = work.tile([P, 2], f32, tag="wd")
            nc.vector.tensor_tensor(wd[:], bx[:, 2:4], bx[:, 0:2],
                                    op=OP.subtract)
            area = work.tile([P, 1], f32, tag="area")
            nc.vector.tensor_tensor(area[:], wd[:, 0:1], wd[:, 1:2],
                                    op=OP.mult)
            return srt, bx, area

        def phase4(img, srt, bx, area):
            """coordinate row broadcasts + IoU suppression matrix."""
            brow = work.tile([P, 5, P], f32, tag="brow")
            for c in range(5):
                csrc = bx[:, c:c + 1] if c < 4 else area[:, 0:1]
                tp = tpsp.tile([1, P], f32, tag="tp")
                nc.tensor.transpose(tp[:], csrc, ident)
                rowc = work.tile([1, P], f32, tag="rowc")
                _acopy(nc, rowc[:], tp[:])
                bp = brpsp.tile([P, P], f32, tag="brps")
                nc.tensor.matmul(bp[:], lhsT=ones1, rhs=rowc[:], start=True,
                                 stop=(c < 4))
                if c == 4:
                    # area row + 1e30 on the i>=j half: S's is_gt then
                    # bakes in the strict-upper mask for free
                    nc.tensor.matmul(bp[:], lhsT=bigid, rhs=lt_incl,
                                     start=False, stop=True)
                _acopy(nc, brow[:, c, :], bp[:])

            xx1 = work.tile([P, P], f32, tag="xx1")
            nc.vector.tensor_scalar(xx1[:], brow[:, 0, :], bx[:, 0:1], None,
                                    op0=OP.max)
            iw = work.tile([P, P], f32, tag="iw")
            nc.vector.scalar_tensor_tensor(iw[:], brow[:, 2, :], bx[:, 2:3],
                                           xx1[:], op0=OP.min, op1=OP.subtract)
            yy1 = work.tile([P, P], f32, tag="yy1")
            nc.vector.tensor_scalar(yy1[:], brow[:, 1, :], bx[:, 1:2], None,
                                    op0=OP.max)
            ih = work.tile([P, P], f32, tag="ih")
            nc.vector.scalar_tensor_tensor(ih[:], brow[:, 3, :], bx[:, 3:4],
                                           yy1[:], op0=OP.min, op1=OP.subtract)
            nc.vector.tensor_scalar(ih[:], ih[:], 0.0, None, op0=OP.max)
            inter = work.tile([P, P], f32, tag="inter")
            nc.vector.scalar_tensor_tensor(inter[:], iw[:], 0.0, ih[:],
                                           op0=OP.max, op1=OP.mult)
            # v1 = 3*inter - area_i ; S = (v1 - eps) > (area_j + 1e30*[i>=j])
            v1 = work.tile([P, P], f32, tag="v1")
            nc.vector.tensor_scalar(v1[:], inter[:], 3.0, area[:, 0:1],
                                    op0=OP.mult, op1=OP.subtract)
            S = work.tile([P, P], f32, tag="S")
            nc.vector.scalar_tensor_tensor(S[:], v1[:], 1e-9, brow[:, 4, :],
                                           op0=OP.subtract, op1=OP.is_gt)
            return srt, bx, S

        def phase5(img, srt, bx, S):
            """Jacobi NMS + prefix compaction + output row."""
            sig = work.tile([P, 1], f32, tag="sig")
            nc.scalar.activation(sig[:], srt[:, 0:1], AF.Exp, scale=-1.0)
            nc.vector.tensor_scalar(sig[:], sig[:], 1.0, None, op0=OP.add)
            nc.vector.reciprocal(sig[:], sig[:])
            K = work.tile([P, 1], f32, tag="keep")
            nc.vector.memset(K[:], 1.0)
            for it in range(NMS_ITERS):
                sup_ps = tailp.tile([P, 1], f32, tag="tail")
                nc.tensor.matmul(sup_ps[:], lhsT=S[:], rhs=K[:], start=True,
                                 stop=True)
                K = work.tile([P, 1], f32, tag="keep")
                nc.vector.tensor_scalar(K[:], sup_ps[:], 0.5, None,
                                        op0=OP.is_lt)

            pref_ps = tailp.tile([P, 1], f32, tag="tail")
            nc.tensor.matmul(pref_ps[:], lhsT=ut_incl, rhs=K[:], start=True,
                             stop=True)
            krank = work.tile([P, 1], f32, tag="krank")
            nc.vector.scalar_tensor_tensor(krank[:], pref_ps[:], 1.0, K[:],
                                           op0=OP.mult, op1=OP.mult)
            nc.vector.tensor_scalar(krank[:], krank[:], 1.0, None,
                                    op0=OP.subtract)
            p2 = work.tile([P, P], f32, tag="p2")
            nc.vector.tensor_scalar(p2[:], iota128c, krank[:, 0:1], None,
                                    op0=OP.is_equal)
            dm = work.tile([P, 5], f32, tag="dm")
            nc.vector.tensor_copy(dm[:, 0:4], bx[:])
            nc.vector.tensor_copy(dm[:, 4:5], sig[:])
            outp_ps = tailp.tile([P, 5], f32, tag="tail")
            nc.tensor.matmul(outp_ps[:], lhsT=p2[:], rhs=dm[:], start=True,
                             stop=True)
            nc.vector.tensor_copy(outall[:, 5 * img:5 * (img + 1)],
                                  outp_ps[:])

        st = [phase1(i) for i in range(IMGS)]
        st = [phase2(i, st[i]) for i in range(IMGS)]
        st = [phase3(i, *st[i]) for i in range(IMGS)]
        for i in range(IMGS):
            phase5(i, *phase4(i, *st[i]))

        nc.sync.dma_start(
            out.rearrange("i p c -> p i c"),
            outall[0:MAXDET, :].rearrange("p (i c) -> p i c", c=5))


def build_program():
    nc = bacc.Bacc("TRN2", target_bir_lowering=False, debug=False,
                   num_swdge_queues=2)
    conf_d = nc.dram_tensor("conf", [IMGS, N], f32, kind="ExternalInput")
    bbox_d = nc.dram_tensor("bbox", [IMGS, N, 4], f32, kind="ExternalInput")
    anch_d = nc.dram_tensor("anchors", [N, 4], f32, kind="ExternalInput")
    cst_d = nc.dram_tensor("cst", [P, CW], f32, kind="ExternalInput")
    out_d = nc.dram_tensor("out", [IMGS, MAXDET, 5], f32, kind="ExternalOutput")
    with tile.TileContext(nc) as tc:
        build_kernel(tc, conf_d.ap(), bbox_d.ap(), anch_d.ap(), cst_d.ap(),
                     out_d.ap())
    nc.compile()
    return nc


_nc_cache = None


def kernel(bbox_pred, conf_pred, anchors):
    global _nc_cache
    from concourse.bass_utils import run_bass_kernel_spmd

    if _nc_cache is None:
        _nc_cache = build_program()
    nc = _nc_cache

    bbox_pred = np.ascontiguousarray(np.asarray(bbox_pred, dtype=np.float32))
    conf_pred = np.ascontiguousarray(np.asarray(conf_pred, dtype=np.float32))
    anchors = np.ascontiguousarray(np.asarray(anchors, dtype=np.float32))
    cst = make_consts()

    in_maps = []
    for c in range(NCORES):
        sl = slice(c * IMGS, (c + 1) * IMGS)
        in_maps.append({
            "conf": conf_pred[sl],
            "bbox": bbox_pred[sl],
            "anchors": anchors,
            "cst": cst,
        })
    res = run_bass_kernel_spmd(nc, in_maps, core_ids=list(range(NCORES)))
    return np.concatenate([res.results[c]["out"] for c in range(NCORES)], axis=0)


# revision 50
# speedup vs baseline: 1716.9552x; 1716.9552x over previous
"""DogDetector NMS kernel for Trainium2 (Bass/Tile), 8-core data-parallel.

Algorithm (per core, 4 images):
  - gpsimd InstTopk: exact top-256 (values + indices) per 51200-anchor
    half-vocab per image (TOPK ISA n field is u16); the global top-128 of
    an image is contained in the union of its two halves' top-128s
  - exact descending sort of that 256-pool by (conf, -index): pairwise
    compare matrix reduced on DVE for ranks, permutation matmul on PE
    (reproduces jax.lax.top_k tie ordering)
  - per image, gather bbox deltas + anchors of the top-128 with 256B-block
    dma_gathers (bbox_pred is never fully read)
  - decode boxes, IoU matrix [128,128], greedy NMS as a Jacobi fixed
    point iterated on PE, prefix-sum compaction matmul -> [100,5]
"""

import sys

import numpy as np

sys.path.insert(0, "/opt/trn_rl_repo")

import concourse.bacc as bacc
import concourse.bass_isa as bass_isa
import concourse.mybir as mybir
import concourse.tile as tile

B = 32
N = 102400
NCORES = 8
IMGS = B // NCORES          # 4 images per core
P = 128
HV = N // 2                 # half-vocab per topk call
POOL = 256                  # candidate pool (2 x topk half top-128)
NMS_ITERS = 4               # Jacobi iterations (empirical max 3)
MAXDET = 100
CLIP = 4.135

f32 = mybir.dt.float32
i32 = mybir.dt.int32
i16 = mybir.dt.int16
u32 = mybir.dt.uint32
OP = mybir.AluOpType
AF = mybir.ActivationFunctionType
AX = mybir.AxisListType

# host-built constant block [128, CW] f32; column layout:
C_ID = 0        # identity [128,128]
C_UTI = 128     # upper-tri incl diag: [i,j]=1 iff i<=j (row 0 = all ones)
C_UTS = 256     # upper-tri strict:    [i,j]=1 iff i<j
C_I128 = 384    # row 0..127 replicated on all partitions
C_I16 = 512     # row 0..15 replicated
C_M16 = 528     # [r, m] = 1 iff r%16 == m%16   (wrapped-index selection)
C_GRP = 656     # [r, c] = 1 iff r//16 == c     (c = 0..7)
C_LTI = 664     # [k, j] = 1 iff k >= j (lower-tri incl)
C_BID = 792     # 1e30 * identity
CW = 920


def make_consts():
    c = np.zeros((P, CW), np.float32)
    i = np.arange(P)
    c[:, C_ID:C_ID + P] = np.eye(P, dtype=np.float32)
    c[:, C_UTI:C_UTI + P] = (i[:, None] <= i[None, :]).astype(np.float32)
    c[:, C_UTS:C_UTS + P] = (i[:, None] < i[None, :]).astype(np.float32)
    c[:, C_I128:C_I128 + P] = i[None, :].astype(np.float32)
    c[:, C_I16:C_I16 + 16] = np.arange(16, dtype=np.float32)[None, :]
    c[:, C_M16:C_M16 + P] = (i[:, None] % 16 == i[None, :] % 16).astype(np.float32)
    c[:, C_GRP:C_GRP + 8] = (i[:, None] // 16 == np.arange(8)[None, :]).astype(
        np.float32)
    c[:, C_LTI:C_LTI + P] = (i[:, None] >= i[None, :]).astype(np.float32)
    c[:, C_BID:C_BID + P] = 1e30 * np.eye(P, dtype=np.float32)
    return c


def _acopy(nc, out, in_):
    nc.scalar.activation(out, in_, AF.Copy)


def build_kernel(tc, conf, bbox, anch, cst, out):
    nc = tc.nc
    with (
        tc.tile_pool(name="consts", bufs=1) as consts,
        tc.tile_pool(name="confp", bufs=1) as confp,
        tc.tile_pool(name="work", bufs=4) as work,
        tc.tile_pool(name="pp", bufs=2, space="PSUM") as pp,
        tc.tile_pool(name="tailp", bufs=1, space="PSUM") as tailp,
        tc.tile_pool(name="tpsp", bufs=2, space="PSUM") as tpsp,
        tc.tile_pool(name="brpsp", bufs=2, space="PSUM") as brpsp,
        tc.tile_pool(name="bb", bufs=1, space="PSUM") as bb,
    ):
        C = consts.tile([P, CW], f32)
        ident = C[:, C_ID:C_ID + P]
        ut_incl = C[:, C_UTI:C_UTI + P]
        ut_strict = C[:, C_UTS:C_UTS + P]
        iota128c = C[:, C_I128:C_I128 + P]
        iota16c = C[:, C_I16:C_I16 + 16]
        mod16eq = C[:, C_M16:C_M16 + P]
        grp8 = C[:, C_GRP:C_GRP + 8]
        lt_incl = C[:, C_LTI:C_LTI + P]
        bigid = C[:, C_BID:C_BID + P]
        ones1 = C[0:1, C_UTI:C_UTI + P]         # row 0 of ut_incl is all ones

        # ---- stage A: topk per half-vocab, batched row layouts ----
        rowv = []   # per half: [1, 1024] pool values (img-major, ascending)
        rowi = []   # per half: [1, 1024] global anchor indices (f32)
        # one topk call, 8 tokens: tokens 0..3 = half A of each image,
        # tokens 4..7 = half B (engages all 8 Q7 cores at once)
        ctile = confp.tile([P, HV // 16], f32, tag="conf")
        for h in range(2):
            nc.sync.dma_start(
                ctile[64 * h:64 * (h + 1), :],
                conf[:, h * HV:(h + 1) * HV].rearrange("i (p c) -> i p c", p=16))
        nc.sync.dma_start(C[:], cst)
        tk = confp.tile([P, 32], u32, tag="tk")
        nc.gpsimd.add_instruction(bass_isa.InstTopk(
            name=f"I-{nc.next_id()}",
            ins=[nc.gpsimd.lower_ap(ctile[:], for_isa=True)],
            outs=[nc.gpsimd.lower_ap(tk[:], for_isa=True)],
            _tokens=2 * IMGS, _n=HV, _k=POOL))
        ti = confp.tile([P, 16], f32, tag="tkif")
        nc.vector.tensor_copy(ti[:], tk[:, 16:32])
        nc.vector.tensor_scalar(ti[64:P, :], ti[64:P, :], float(HV), None,
                                op0=OP.add)
        for h in range(2):
            rv = confp.tile([1, 1024], f32, tag=f"rowv{h}")
            nc.sync.dma_start(rv[:], tk[64 * h:64 * (h + 1), 0:16].bitcast(f32))
            ri = confp.tile([1, 1024], f32, tag=f"rowi{h}")
            nc.sync.dma_start(ri[:], ti[64 * h:64 * (h + 1), :])
            rowv.append(rv)
            rowi.append(ri)

        outall = work.tile([P, 5 * IMGS], f32, tag="outall")

        # ---- phase-major emission: all images phase 1, then phase 2, ... ----
        def phase1(img):
            """rows bcast + vi columns + exact ranks + permutation sort."""
            sl = slice(POOL * img + P, POOL * img + 2 * P)
            rA, rB = rowv[0][0:1, sl], rowv[1][0:1, sl]
            iA, iB = rowi[0][0:1, sl], rowi[1][0:1, sl]

            vrow_bm = work.tile([P, POOL], f32, tag="vrbm")
            irow_bm = work.tile([P, POOL], f32, tag="irbm")
            for dst, (sa, sb) in ((vrow_bm, (rA, rB)), (irow_bm, (iA, iB))):
                t_ps = bb.tile([P, POOL], f32, tag="bb")
                nc.tensor.matmul(t_ps[:, 0:P], lhsT=ones1, rhs=sa, start=True,
                                 stop=True)
                nc.tensor.matmul(t_ps[:, P:POOL], lhsT=ones1, rhs=sb,
                                 start=True, stop=True)
                _acopy(nc, dst[:], t_ps[:])

            vi = work.tile([P, 4], f32, tag="vi")   # val A, val B, idx A, idx B
            vit_ps = pp.tile([P, 4], f32, tag="pp")
            for col, rsrc in enumerate((rA, rB, iA, iB)):
                nc.tensor.transpose(vit_ps[:, col:col + 1], rsrc,
                                    ident[0:1, 0:1])
            _acopy(nc, vi[:], vit_ps[:])

            rcol = work.tile([P, 2], f32, tag="rcol")
            for blk in range(2):
                rg = work.tile([P, 2], f32, tag="rg")   # gt count, tie count
                a = work.tile([P, POOL], f32, tag="rgt")
                nc.vector.tensor_scalar(a[:], vrow_bm[:], vi[:, blk:blk + 1],
                                        0.0, op0=OP.is_gt, op1=OP.add,
                                        accum_out=rg[:, 0:1])
                b = work.tile([P, POOL], f32, tag="req")
                nc.vector.tensor_scalar(b[:], vrow_bm[:], vi[:, blk:blk + 1],
                                        None, op0=OP.is_equal)
                c_ = work.tile([P, POOL], f32, tag="ril")
                nc.vector.scalar_tensor_tensor(
                    c_[:], irow_bm[:], vi[:, 2 + blk:3 + blk], b[:],
                    op0=OP.is_lt, op1=OP.mult, accum_out=rg[:, 1:2])
                nc.vector.tensor_tensor(rcol[:, blk:blk + 1], rg[:, 0:1],
                                        rg[:, 1:2], op=OP.add)

            srt_ps = pp.tile([P, 2], f32, tag="pp")
            for blk in range(2):
                pt = work.tile([P, P], f32, tag="pt")
                nc.vector.tensor_scalar(pt[:], iota128c, rcol[:, blk:blk + 1],
                                        None, op0=OP.is_equal)
                nc.tensor.matmul(srt_ps[:], lhsT=pt[:], rhs=vi[:, blk:4:2],
                                 start=(blk == 0), stop=(blk == 1))
            srt = work.tile([P, 2], f32, tag="srt")
            nc.vector.tensor_copy(srt[:], srt_ps[:])
            return srt

        def phase2(img, srt):
            """gather indices + the two dma_gathers."""
            ai = work.tile([P, 1], i32, tag="ai")
            nc.vector.tensor_copy(ai[:], srt[:, 1:2])
            blocki = work.tile([P, 1], i32, tag="blocki")
            nc.vector.tensor_scalar(blocki[:], ai[:], 4, None,
                                    op0=OP.logical_shift_right)
            withini = work.tile([P, 1], i32, tag="withini")
            nc.vector.tensor_scalar(withini[:], ai[:], 15, None,
                                    op0=OP.bitwise_and)
            withinf = work.tile([P, 1], f32, tag="withinf")
            nc.vector.tensor_copy(withinf[:], withini[:])
            blockf = work.tile([P, 1], f32, tag="blockf")
            nc.vector.tensor_copy(blockf[:], blocki[:])

            rhs8 = work.tile([P, 8], f32, tag="rhs8")
            nc.vector.tensor_tensor(rhs8[:], blockf[:].broadcast_to([P, 8]),
                                    grp8, op=OP.mult)
            wrap_ps = pp.tile([P, 8], f32, tag="pp")
            nc.tensor.matmul(wrap_ps[:], lhsT=mod16eq, rhs=rhs8[:], start=True,
                             stop=True)
            widx = work.tile([P, 8], i16, tag="widx")
            nc.vector.tensor_copy(widx[:], wrap_ps[:])

            gbox = work.tile([P, 1, 64], f32, tag="gbox")
            nc.gpsimd.dma_gather(
                gbox[:], bbox[img].rearrange("(b k) c -> b (k c)", k=16),
                widx[:], num_idxs=P, num_idxs_reg=P, elem_size=64,
                queue_num=img % 2)
            ganch = work.tile([P, 1, 64], f32, tag="ganch")
            nc.gpsimd.dma_gather(
                ganch[:], anch.rearrange("(b k) c -> b (k c)", k=16),
                widx[:], num_idxs=P, num_idxs_reg=P, elem_size=64,
                queue_num=(img + 1) % 2)
            return srt, withinf, gbox, ganch

        def phase3(img, srt, withinf, gbox, ganch):
            """extract gathered rows + box decode."""
            em = work.tile([P, 16], f32, tag="em")
            nc.vector.tensor_scalar(em[:], iota16c, withinf[:, 0:1], None,
                                    op0=OP.is_equal)
            dd = {}
            for key, gt_ in (("d", gbox), ("a", ganch)):
                pb = work.tile([P, 16, 4], f32, tag="pb")
                nc.vector.tensor_tensor(
                    pb[:], gt_[:, 0, :].rearrange("p (b c) -> p b c", c=4),
                    em[:].rearrange("p (b o) -> p b o", o=1)
                        .broadcast_to([P, 16, 4]),
                    op=OP.mult)
                t4 = work.tile([P, 4], f32, tag=f"ex{key}")
                nc.vector.tensor_reduce(t4[:],
                                        pb[:].rearrange("p b c -> p c b"),
                                        axis=AX.X, op=OP.add)
                dd[key] = t4
            dl, aa = dd["d"], dd["a"]

            wh = work.tile([P, 2], f32, tag="wh")
            nc.vector.tensor_tensor(wh[:], aa[:, 2:4], aa[:, 0:2],
                                    op=OP.subtract)
            ctr = work.tile([P, 2], f32, tag="ctr")
            nc.vector.tensor_scalar(ctr[:], wh[:], 0.5, None, op0=OP.mult)
            nc.vector.tensor_tensor(ctr[:], ctr[:], aa[:, 0:2], op=OP.add)
            dwh = work.tile([P, 2], f32, tag="dwh")
            nc.vector.tensor_scalar(dwh[:], dl[:, 2:4], -CLIP, CLIP,
                                    op0=OP.max, op1=OP.min)
            ewh = work.tile([P, 2], f32, tag="ewh")
            nc.scalar.activation(ewh[:], dwh[:], AF.Exp)
            pwh = work.tile([P, 2], f32, tag="pwh")
            nc.vector.tensor_tensor(pwh[:], ewh[:], wh[:], op=OP.mult)
            pc = work.tile([P, 2], f32, tag="pc")
            nc.vector.tensor_tensor(pc[:], dl[:, 0:2], wh[:], op=OP.mult)
            nc.vector.tensor_tensor(pc[:], pc[:], ctr[:], op=OP.add)
            hwh = work.tile([P, 2], f32, tag="hwh")
            nc.vector.tensor_scalar(hwh[:], pwh[:], 0.5, None, op0=OP.mult)
            bx = work.tile([P, 4], f32, tag="bx")
            nc.vector.tensor_tensor(bx[:, 0:2], pc[:], hwh[:], op=OP.subtract)
            nc.vector.tensor_tensor(bx[:, 2:4], pc[:], hwh[:], op=OP.add)
            wd = work.tile([P, 2], f32, tag="wd")
            nc.vector.tensor_tensor(wd[:], bx[:, 2:4], bx[:, 0:2],
                                    op=OP.subtract)
            area = work.tile([P, 1], f32, tag="area")
            nc.vector.tensor_tensor(area[:], wd[:, 0:1], wd[:, 1:2],
                                    op=OP.mult)
            return srt, bx, area

        def phase4(img, srt, bx, area):
            """coordinate row broadcasts + IoU suppression matrix."""
            brow = work.tile([P, 5, P], f32, tag="brow")
            for c in range(5):
                csrc = bx[:, c:c + 1] if c < 4 else area[:, 0:1]
                tp = tpsp.tile([1, P], f32, tag="tp")
                nc.tensor.transpose(tp[:], csrc, ident)
                rowc = work.tile([1, P], f32, tag="rowc")
                _acopy(nc, rowc[:], tp[:])
                bp = brpsp.tile([P, P], f32, tag="brps")
                nc.tensor.matmul(bp[:], lhsT=ones1, rhs=rowc[:], start=True,
                                 stop=(c < 4))
                if c == 4:
                    # area row + 1e30 on the i>=j half: S's is_gt then
                    # bakes in the strict-upper mask for free
                    nc.tensor.matmul(bp[:], lhsT=bigid, rhs=lt_incl,
                                     start=False, stop=True)
                _acopy(nc, brow[:, c, :], bp[:])

            xx1 = work.tile([P, P], f32, tag="xx1")
            nc.vector.tensor_scalar(xx1[:], brow[:, 0, :], bx[:, 0:1], None,
                                    op0=OP.max)
            iw = work.tile([P, P], f32, tag="iw")
            nc.vector.scalar_tensor_tensor(iw[:], brow[:, 2, :], bx[:, 2:3],
                                           xx1[:], op0=OP.min, op1=OP.subtract)
            yy1 = work.tile([P, P], f32, tag="yy1")
            nc.vector.tensor_scalar(yy1[:], brow[:, 1, :], bx[:, 1:2], None,
                                    op0=OP.max)
            ih = work.tile([P, P], f32, tag="ih")
            nc.vector.scalar_tensor_tensor(ih[:], brow[:, 3, :], bx[:, 3:4],
                                           yy1[:], op0=OP.min, op1=OP.subtract)
            nc.vector.tensor_scalar(ih[:], ih[:], 0.0, None, op0=OP.max)
            inter = work.tile([P, P], f32, tag="inter")
            nc.vector.scalar_tensor_tensor(inter[:], iw[:], 0.0, ih[:],
                                           op0=OP.max, op1=OP.mult)
            # v1 = 3*inter - area_i ; S = (v1 - eps) > (area_j + 1e30*[i>=j])
            v1 = work.tile([P, P], f32, tag="v1")
            nc.vector.tensor_scalar(v1[:], inter[:], 3.0, area[:, 0:1],
                                    op0=OP.mult, op1=OP.subtract)
            S = work.tile([P, P], f32, tag="S")
            nc.vector.scalar_tensor_tensor(S[:], v1[:], 1e-9, brow[:, 4, :],
                                           op0=OP.subtract, op1=OP.is_gt)
            return srt, bx, S

        def phase5(img, srt, bx, S):
            """Jacobi NMS + prefix compaction + output row."""
            sig = work.tile([P, 1], f32, tag="sig")
            nc.scalar.activation(sig[:], srt[:, 0:1], AF.Exp, scale=-1.0)
            nc.vector.tensor_scalar(sig[:], sig[:], 1.0, None, op0=OP.add)
            nc.vector.reciprocal(sig[:], sig[:])
            K = work.tile([P, 1], f32, tag="keep")
            nc.vector.memset(K[:], 1.0)
            for it in range(NMS_ITERS):
                sup_ps = tailp.tile([P, 1], f32, tag="tail")
                nc.tensor.matmul(sup_ps[:], lhsT=S[:], rhs=K[:], start=True,
                                 stop=True)
                K = work.tile([P, 1], f32, tag="keep")
                nc.vector.tensor_scalar(K[:], sup_ps[:], 0.5, None,
                                        op0=OP.is_lt)

            pref_ps = tailp.tile([P, 1], f32, tag="tail")
            nc.tensor.matmul(pref_ps[:], lhsT=ut_incl, rhs=K[:], start=True,
                             stop=True)
            krank = work.tile([P, 1], f32, tag="krank")
            nc.vector.scalar_tensor_tensor(krank[:], pref_ps[:], 1.0, K[:],
                                           op0=OP.mult, op1=OP.mult)
            nc.vector.tensor_scalar(krank[:], krank[:], 1.0, None,
                                    op0=OP.subtract)
            p2 = work.tile([P, P], f32, tag="p2")
            nc.vector.tensor_scalar(p2[:], iota128c, krank[:, 0:1], None,
                                    op0=OP.is_equal)
            dm = work.tile([P, 5], f32, tag="dm")
            nc.vector.tensor_copy(dm[:, 0:4], bx[:])
            nc.vector.tensor_copy(dm[:, 4:5], sig[:])
            outp_ps = tailp.tile([P, 5], f32, tag="tail")
            nc.tensor.matmul(outp_ps[:], lhsT=p2[:], rhs=dm[:], start=True,
                             stop=True)
            nc.vector.tensor_copy(outall[:, 5 * img:5 * (img + 1)],
                                  outp_ps[:])

        st = [phase1(i) for i in range(IMGS)]
        st = [phase2(i, st[i]) for i in range(IMGS)]
        st = [phase3(i, *st[i]) for i in range(IMGS)]
        for i in range(IMGS):
            phase5(i, *phase4(i, *st[i]))

        nc.sync.dma_start(
            out.rearrange("i p c -> p i c"),
            outall[0:MAXDET, :].rearrange("p (i c) -> p i c", c=5))


def build_program():
    nc = bacc.Bacc("TRN2", target_bir_lowering=False, debug=False,
                   num_swdge_queues=2)
    conf_d = nc.dram_tensor("conf", [IMGS, N], f32, kind="ExternalInput")
    bbox_d = nc.dram_tensor("bbox", [IMGS, N, 4], f32, kind="ExternalInput")
    anch_d = nc.dram_tensor("anchors", [N, 4], f32, kind="ExternalInput")
    cst_d = nc.dram_tensor("cst", [P, CW], f32, kind="ExternalInput")
    out_d = nc.dram_tensor("out", [IMGS, MAXDET, 5], f32, kind="ExternalOutput")
    with tile.TileContext(nc) as tc:
        build_kernel(tc, conf_d.ap(), bbox_d.ap(), anch_d.ap(), cst_d.ap(),
                     out_d.ap())
    nc.compile()
    return nc


_nc_cache = None


def kernel(bbox_pred, conf_pred, anchors):
    global _nc_cache
    from concourse.bass_utils import run_bass_kernel_spmd

    if _nc_cache is None:
        _nc_cache = build_program()
    nc = _nc_cache

    bbox_pred = np.ascontiguousarray(np.asarray(bbox_pred, dtype=np.float32))
    conf_pred = np.ascontiguousarray(np.asarray(conf_pred, dtype=np.float32))
    anchors = np.ascontiguousarray(np.asarray(anchors, dtype=np.float32))
    cst = make_consts()

    in_maps = []
    for c in range(NCORES):
        sl = slice(c * IMGS, (c + 1) * IMGS)
        in_maps.append({
            "conf": conf_pred[sl],
            "bbox": bbox_pred[sl],
            "anchors": anchors,
            "cst": cst,
        })
    res = run_bass_kernel_spmd(nc, in_maps, core_ids=list(range(NCORES)))
    return np.concatenate([res.results[c]["out"] for c in range(NCORES)], axis=0)
